# revision 16
# baseline (speedup 1.0000x reference)
"""Trainium2 Bass kernel for nn_CNN_pre_LSTM (dense_cnn).

Reference computation per sample (L=24):
    h = relu(conv1d(x, w11, b11))    # 1 -> 8 ch, k=3, same pad
    h = relu(conv1d(h, w12, b12))    # 8 -> 8
    h = maxpool2(h)                  # L 24 -> 12
    h = relu(conv1d(h, w21, b21))    # 8 -> 16
    h = relu(conv1d(h, w22, b22))    # 16 -> 16
    h = maxpool2(h)                  # L 12 -> 6
    y = h.reshape(96) @ Wl.T + bl    # 96 -> 24

Mapping: pure data parallel over the fused (S*B) batch across 8 cores;
16384 samples per core. On chip, activations live as [feature, batch_tile]
(features on SBUF partitions, batch on the free dim) and every conv layer
is a small set of dense banded matmuls built on the host:

  - features are ordered l-major/c-minor; non-pooled layers emit two
    l-halves with a halo overlap so the next layer's contraction window is
    a single SBUF tile (one logical matmul per block, no PSUM
    accumulation anywhere).
  - pooled layers (conv12, conv22) emit one PSUM tensor with all EVEN
    output positions and one with all ODD positions (two col-tiled
    matmuls per tensor, at array column offsets 0 and 64, which the PE
    runs concurrently). The maxpool is then a single partition-aligned
    elementwise max of the two relu'd tensors.
  - maxpool commutes with relu and the per-channel bias add, so bias+relu
    are applied during PSUM evacuation (ACT activation with per-partition
    bias, or DVE tensor_scalar (x+bias) max 0), and the pool runs on the
    evacuated SBUF tiles (GPSIMD tensor_max, keeping DVE free).

The input is pre-transposed/chunked on the host to [n_tiles, 24, NT] per
core (DRAM partition strides must stay <= 32KB), and the output is
produced as [n_tiles, 24, NT] and reassembled on the host. All weights
and biases ship as two packed blobs (one DMA each at kernel start).
"""

import numpy as np

import concourse.bass as bass
import concourse.tile as tile
import concourse.mybir as mybir
from concourse import bacc
from concourse.bass_utils import run_bass_kernel_spmd

# ---------------------------------------------------------------- config
N_CORES = 8
S, B, L = 512, 256, 24
SB = S * B
CORE_N = SB // N_CORES  # 16384

# compute dtype for matmul operands / intermediate activations:
#   "fp16"  : float16 operands, fp32 PSUM accumulate, NT=1024
#   "fp32r" : fp32 bits, PE in float32r mode (full rate at N>=256), NT=512
#   "fp32"  : exact fp32 (PE 4x slower), NT=512
COMPUTE = "fp16"


def _cfg(compute):
    if compute == "fp16":
        return dict(dt=mybir.dt.float16, np_dt=np.float16, nt=1024, mm_cast=None)
    if compute == "fp32r":
        return dict(
            dt=mybir.dt.float32, np_dt=np.float32, nt=512, mm_cast=mybir.dt.float32r
        )
    if compute == "fp32":
        return dict(dt=mybir.dt.float32, np_dt=np.float32, nt=512, mm_cast=None)
    raise ValueError(compute)


# ------------------------------------------------- host weight transforms
#
# Feature row orderings (all l-major, c-minor):
#   h1 block A: rows (l, c)  l in [0,13), c in [0,8)   -> 104 rows
#   h1 block B: rows (l, c)  l in [11,24)              -> 104 rows
#   pooled h2:  rows [lp 0..5 x8ch | 16 pad | lp 6..11 x8ch] = 112
#   h3 block A: rows (l, c16) l in [0,7)               -> 112 rows
#   h3 block B: rows (l-5, c16) l in [5,12)            -> 112 rows
#   pooled h4:  rows [lp 0..2 x16ch | 16 pad | lp 3..5 x16ch] = 112
#   out: rows j in [0,24)
#
# Pooled-layer PSUM tensors (parity split): psP = even l_out, psQ = odd,
# each with rows [first-half (48) | 16 pad | second-half (48)]; pool =
# elementwise max(relu(psP+b), relu(psQ+b)).

def _band_first(w, l_ins, l_outs, cin, cout):
    """Dense banded matrix [len(l_ins)*cin, len(l_outs)*cout] for a k=3
    'same' conv, rows (l_in, ci) l-major, cols (l_out, co) l-major."""
    K = len(l_ins) * cin
    M = len(l_outs) * cout
    W = np.zeros((K, M), np.float32)
    for ki, li in enumerate(l_ins):
        for ci in range(cin):
            for mo, lo in enumerate(l_outs):
                d = li - lo + 1
                if 0 <= d < 3:
                    for co in range(cout):
                        W[ki * cin + ci, mo * cout + co] = w[co, ci, d]
    return W


def _pad48(W):
    """Insert 16 zero rows at row 48 (pooled tensors carry a pad block)."""
    return np.concatenate([W[:48], np.zeros((16,) + W.shape[1:], W.dtype), W[48:]], 0)


def _host_weights(w11, b11, w12, b12, w21, b21, w22, b22, Wl, bl):
    f32 = np.float32
    w11, w12, w21, w22, Wl = (np.asarray(a, f32) for a in (w11, w12, w21, w22, Wl))

    W11A = _band_first(w11, range(0, 24), range(0, 13), 1, 8)      # [24, 104]
    W11B = _band_first(w11, range(0, 24), range(11, 24), 1, 8)     # [24, 104]
    # conv12 parity-split: half A covers l_out in [0,12) (from h1 block A),
    # half B covers [12,24) (from h1 block B)
    W12AE = _band_first(w12, range(0, 13), range(0, 12, 2), 8, 8)    # [104, 48]
    W12AO = _band_first(w12, range(0, 13), range(1, 12, 2), 8, 8)    # [104, 48]
    W12BE = _band_first(w12, range(11, 24), range(12, 24, 2), 8, 8)  # [104, 48]
    W12BO = _band_first(w12, range(11, 24), range(13, 24, 2), 8, 8)  # [104, 48]
    W21A = _pad48(_band_first(w21, range(0, 12), range(0, 7), 8, 16))   # [112, 112]
    W21B = _pad48(_band_first(w21, range(0, 12), range(5, 12), 8, 16))  # [112, 112]
    W22AE = _band_first(w22, range(0, 7), range(0, 6, 2), 16, 16)    # [112, 48]
    W22AO = _band_first(w22, range(0, 7), range(1, 6, 2), 16, 16)    # [112, 48]
    W22BE = _band_first(w22, range(5, 12), range(6, 12, 2), 16, 16)  # [112, 48]
    W22BO = _band_first(w22, range(5, 12), range(7, 12, 2), 16, 16)  # [112, 48]
    # torch flatten feature = c*6 + lp ; h4 row = lp*16 + c (plus pad48)
    WLIN = np.zeros((96, 24), f32)
    for lp in range(6):
        for c in range(16):
            WLIN[lp * 16 + c, :] = Wl[:, c * 6 + lp]
    WLIN = _pad48(WLIN)                                            # [112, 24]

    # pad even halves to 64 cols so the matmul also zeroes rows 48:64 of
    # the [112] psum tensors (initializes the pad block)
    W12AE = np.concatenate([W12AE, np.zeros((104, 16), f32)], 1)   # [104, 64]
    W12AO = np.concatenate([W12AO, np.zeros((104, 16), f32)], 1)
    W22AE = np.concatenate([W22AE, np.zeros((112, 16), f32)], 1)   # [112, 64]
    W22AO = np.concatenate([W22AO, np.zeros((112, 16), f32)], 1)

    return {
        "w11a": W11A, "w11b": W11B,
        "w12ae": W12AE, "w12ao": W12AO, "w12be": W12BE, "w12bo": W12BO,
        "w21a": W21A, "w21b": W21B,
        "w22ae": W22AE, "w22ao": W22AO, "w22be": W22BE, "w22bo": W22BO,
        "wlin": WLIN,
        "b11v": np.tile(np.asarray(b11, f32), 13).reshape(104, 1),
        "b12v": np.tile(np.asarray(b12, f32), 14).reshape(112, 1),
        "b21v": np.tile(np.asarray(b21, f32), 7).reshape(112, 1),
        "b22v": np.tile(np.asarray(b22, f32), 7).reshape(112, 1),
        "blv": np.asarray(bl, f32).reshape(24, 1),
    }


# weight blob layout: (name, K, M) in packing order
_WSPEC = [
    ("w11a", 24, 104), ("w11b", 24, 104),
    ("w12ae", 104, 64), ("w12ao", 104, 64),
    ("w12be", 104, 48), ("w12bo", 104, 48),
    ("w21a", 112, 112), ("w21b", 112, 112),
    ("w22ae", 112, 64), ("w22ao", 112, 64),
    ("w22be", 112, 48), ("w22bo", 112, 48),
    ("wlin", 112, 24),
]
_WOFF = {}
_off = 0
for _n, _k, _m in _WSPEC:
    _WOFF[_n] = _off
    _off += _m
WBLOB_COLS = _off  # 904

_BSPEC = [("b11v", 104), ("b12v", 112), ("b21v", 112), ("b22v", 112), ("blv", 24)]
_BOFF = {n: i for i, (n, _) in enumerate(_BSPEC)}


def _pack_blobs(W, np_dt):
    wb = np.zeros((128, WBLOB_COLS), np_dt)
    for n, k, m in _WSPEC:
        assert W[n].shape == (k, m), (n, W[n].shape)
        wb[:k, _WOFF[n]:_WOFF[n] + m] = W[n].astype(np_dt)
    bb = np.zeros((128, len(_BSPEC)), np.float32)
    for n, p in _BSPEC:
        bb[:p, _BOFF[n]] = W[n][:, 0]
    return wb, bb


# ----------------------------------------------------- numpy device model
def emulate(x, np_dt=np.float16, **kw):
    """Pure-numpy emulation of the device dataflow (same banded matrices,
    same orderings, same cast points). Used to validate index math."""
    W = _host_weights(**kw)
    xt = np.ascontiguousarray(x.reshape(-1, L).T).astype(np_dt)  # [24, N]
    n = xt.shape[1]
    c = lambda a: a.astype(np_dt)

    def mm(wname, act):
        return c(W[wname]).astype(np.float32).T @ act.astype(np.float32)

    def relu_b(a, bias):
        return np.maximum(a + bias, 0.0)

    def parity_ps(wa, wb, ha, hb):
        ps = np.zeros((112, n), np.float32)
        ps[0:64] = mm(wa, ha)
        ps[64:112] = mm(wb, hb)
        return c(ps)

    psA, psB = c(mm("w11a", xt)), c(mm("w11b", xt))
    h1a, h1b = c(relu_b(psA, W["b11v"])), c(relu_b(psB, W["b11v"]))
    psP = parity_ps("w12ae", "w12be", h1a, h1b)
    psQ = parity_ps("w12ao", "w12bo", h1a, h1b)
    sP, sQ = c(relu_b(psP, W["b12v"])), c(relu_b(psQ, W["b12v"]))
    h2r = np.maximum(sP, sQ)
    psE, psF = c(mm("w21a", h2r)), c(mm("w21b", h2r))
    h3a, h3b = c(relu_b(psE, W["b21v"])), c(relu_b(psF, W["b21v"]))
    psR = parity_ps("w22ae", "w22be", h3a, h3b)
    psS = parity_ps("w22ao", "w22bo", h3a, h3b)
    sR, sS = c(relu_b(psR, W["b22v"])), c(relu_b(psS, W["b22v"]))
    h4r = np.maximum(sR, sS)
    out = mm("wlin", h4r) + W["blv"]  # fp32
    return out.T.reshape(x.shape[0], x.shape[1], 24).astype(np.float32)


# --------------------------------------------------------- device builder
def build_kernel(n_samples, compute=COMPUTE, n_cores=N_CORES):
    cfg = _cfg(compute)
    DT, NT = cfg["dt"], cfg["nt"]
    MMC = cfg["mm_cast"]
    f32 = mybir.dt.float32
    n_tiles = n_samples // NT
    assert n_samples % NT == 0

    nc = bacc.Bacc(
        "TRN2",
        target_bir_lowering=False,
        debug=False,
        enable_asserts=False,
        num_devices=n_cores,
    )

    # tile-chunked 3D input layout keeps DRAM partition strides small
    # (a flat [24, 16384] fp32 needs 64KB strides, which crash the device)
    xt_d = nc.dram_tensor("xt", [n_tiles, 24, NT], DT, kind="ExternalInput").ap()
    wb_d = nc.dram_tensor("wblob", [128, WBLOB_COLS], DT, kind="ExternalInput").ap()
    bb_d = nc.dram_tensor("bblob", [128, len(_BSPEC)], f32,
                          kind="ExternalInput").ap()
    out_d = nc.dram_tensor("out", [n_tiles, 24, NT], f32, kind="ExternalOutput").ap()

    Relu = mybir.ActivationFunctionType.Relu
    Ident = mybir.ActivationFunctionType.Identity
    Add, Max = mybir.AluOpType.add, mybir.AluOpType.max

    def mmop(ap):
        return ap.bitcast(MMC) if MMC is not None else ap

    # matmul fp32 PSUM output must stay inside one 2KB bank -> <=512 cols
    MMN = min(NT, 512)

    with tile.TileContext(nc) as tc:
        with (
            tc.tile_pool(name="consts", bufs=1) as cpool,
            tc.tile_pool(name="xin", bufs=3) as xpool,
            tc.tile_pool(name="acts", bufs=2) as apool,
            tc.tile_pool(name="outs", bufs=2) as opool,
            tc.tile_pool(name="ps", bufs=3, space="PSUM") as pspool,
            tc.tile_pool(name="pslin", bufs=1, space="PSUM") as lpool,
        ):
            wsb = cpool.tile([128, WBLOB_COLS], DT, tag="wblob")
            bsb = cpool.tile([128, len(_BSPEC)], f32, tag="bblob")
            nc.sync.dma_start(wsb[:], wb_d)
            nc.sync.dma_start(bsb[:], bb_d)

            def w(name):
                k, m = next((kk, mm_) for nn, kk, mm_ in _WSPEC if nn == name)
                return mmop(wsb[0:k, _WOFF[name]:_WOFF[name] + m])

            def bias(name):
                p = next(pp for nn, pp in _BSPEC if nn == name)
                return bsb[0:p, _BOFF[name]:_BOFF[name] + 1]

            def mm(out_ps, wname, rhs_sb, rows=None, tile_pos=None):
                o = out_ps if rows is None else out_ps[rows[0]:rows[1], :]
                for j in range(0, NT, MMN):
                    nc.tensor.matmul(o[:, j:j + MMN], w(wname),
                                     mmop(rhs_sb[:, j:j + MMN]),
                                     start=True, stop=True,
                                     tile_position=tile_pos)

            for t in range(n_tiles):
                xt_t = xpool.tile([24, NT], DT, tag="xt")
                nc.sync.dma_start(xt_t[:], xt_d[t])

                # conv11: two banded matmuls, K=24; ACT evacuates
                psA = pspool.tile([104, NT], f32, tag="ps")
                psB = pspool.tile([104, NT], f32, tag="ps")
                mm(psA, "w11a", xt_t)
                mm(psB, "w11b", xt_t)
                h1a = apool.tile([104, NT], DT, tag="h1a")
                h1b = apool.tile([104, NT], DT, tag="h1b")
                nc.scalar.activation(h1a[:], psA[:], Relu, bias=bias("b11v"))
                nc.scalar.activation(h1b[:], psB[:], Relu, bias=bias("b11v"))

                # conv12: parity-split col-tiled matmuls -> psP (even l),
                # psQ (odd l); ACT evac with bias+relu; pool = one aligned
                # max on GPSIMD
                psP = pspool.tile([112, NT], f32, tag="ps")
                psQ = pspool.tile([112, NT], f32, tag="ps")
                mm(psP, "w12ae", h1a, rows=(0, 64), tile_pos=(0, 0))
                mm(psP, "w12be", h1b, rows=(64, 112), tile_pos=(0, 64))
                mm(psQ, "w12ao", h1a, rows=(0, 64), tile_pos=(0, 0))
                mm(psQ, "w12bo", h1b, rows=(64, 112), tile_pos=(0, 64))
                sP = apool.tile([112, NT], DT, tag="sP")
                sQ = apool.tile([112, NT], DT, tag="sQ")
                nc.scalar.activation(sP[:], psP[:], Relu, bias=bias("b12v"))
                nc.scalar.activation(sQ[:], psQ[:], Relu, bias=bias("b12v"))
                h2r = apool.tile([112, NT], DT, tag="h2r")
                nc.vector.tensor_max(h2r[:], sP[:], sQ[:])

                # conv21: overlapped l-halves; DVE evacuates (bias+relu via
                # tensor_scalar) to balance ACT
                psE = pspool.tile([112, NT], f32, tag="ps")
                psF = pspool.tile([112, NT], f32, tag="ps")
                mm(psE, "w21a", h2r)
                mm(psF, "w21b", h2r)
                h3a = apool.tile([112, NT], DT, tag="h3a")
                h3b = apool.tile([112, NT], DT, tag="h3b")
                nc.vector.tensor_scalar(h3a[:], psE[:], bias("b21v"), 0.0, Add, Max)
                nc.vector.tensor_scalar(h3b[:], psF[:], bias("b21v"), 0.0, Add, Max)

                # conv22: parity-split like conv12; DVE evac; GPSIMD pool
                psR = pspool.tile([112, NT], f32, tag="ps")
                psS = pspool.tile([112, NT], f32, tag="ps")
                mm(psR, "w22ae", h3a, rows=(0, 64), tile_pos=(0, 0))
                mm(psR, "w22be", h3b, rows=(64, 112), tile_pos=(0, 64))
                mm(psS, "w22ao", h3a, rows=(0, 64), tile_pos=(0, 0))
                mm(psS, "w22bo", h3b, rows=(64, 112), tile_pos=(0, 64))
                sR = apool.tile([112, NT], DT, tag="sR")
                sS = apool.tile([112, NT], DT, tag="sS")
                nc.vector.tensor_scalar(sR[:], psR[:], bias("b22v"), 0.0, Add, Max)
                nc.vector.tensor_scalar(sS[:], psS[:], bias("b22v"), 0.0, Add, Max)
                h4r = apool.tile([112, NT], DT, tag="h4r")
                nc.vector.tensor_max(h4r[:], sR[:], sS[:])

                # linear 96 -> 24, fp32 out; ACT evac with bias
                psI = lpool.tile([24, NT], f32, tag="pslin")
                mm(psI, "wlin", h4r)
                osb = opool.tile([24, NT], f32, tag="osb")
                nc.scalar.activation(osb[:], psI[:], Ident, bias=bias("blv"))
                nc.sync.dma_start(out_d[t], osb[:])

    nc.compile()
    return nc


# ------------------------------------------------------------- entry point
def _prep_in_maps(x, weights, compute=COMPUTE):
    cfg = _cfg(compute)
    np_dt = cfg["np_dt"]
    nt = cfg["nt"]
    W = _host_weights(**weights)
    wb, bb = _pack_blobs(W, np_dt)
    xt = np.ascontiguousarray(x.reshape(SB, L).T).astype(np_dt)  # [24, SB]
    in_maps = []
    for c in range(N_CORES):
        xc = xt[:, c * CORE_N:(c + 1) * CORE_N]  # [24, CORE_N]
        in_maps.append({
            "xt": np.ascontiguousarray(
                xc.reshape(24, CORE_N // nt, nt).transpose(1, 0, 2)
            ),
            "wblob": wb,
            "bblob": bb,
        })
    return in_maps


def kernel(x, w11, b11, w12, b12, w21, b21, w22, b22, Wl, bl):
    weights = dict(w11=w11, b11=b11, w12=w12, b12=b12, w21=w21, b21=b21,
                   w22=w22, b22=b22, Wl=Wl, bl=bl)
    x = np.asarray(x, np.float32)
    nc = build_kernel(CORE_N, COMPUTE)
    in_maps = _prep_in_maps(x, weights, COMPUTE)
    res = run_bass_kernel_spmd(nc, in_maps, list(range(N_CORES)))
    outs = [
        res.results[c]["out"].transpose(1, 0, 2).reshape(24, CORE_N)
        for c in range(N_CORES)
    ]
    full = np.concatenate(outs, axis=1)  # [24, SB]
    return np.ascontiguousarray(full.T).reshape(S, B, 24).astype(np.float32)


# revision 18
# speedup vs baseline: 1.0535x; 1.0535x over previous
"""Trainium2 Bass kernel for nn_CNN_pre_LSTM (dense_cnn).

Reference computation per sample (L=24):
    h = relu(conv1d(x, w11, b11))    # 1 -> 8 ch, k=3, same pad
    h = relu(conv1d(h, w12, b12))    # 8 -> 8
    h = maxpool2(h)                  # L 24 -> 12
    h = relu(conv1d(h, w21, b21))    # 8 -> 16
    h = relu(conv1d(h, w22, b22))    # 16 -> 16
    h = maxpool2(h)                  # L 12 -> 6
    y = h.reshape(96) @ Wl.T + bl    # 96 -> 24

Mapping: pure data parallel over the fused (S*B) batch across 8 cores;
16384 samples per core. On chip, activations live as [feature, batch_tile]
(features on SBUF partitions, batch on the free dim) and every conv layer
is a small set of dense banded matmuls built on the host:

  - features are ordered l-major/c-minor; non-pooled layers emit two
    l-halves with a halo overlap so the next layer's contraction window is
    a single SBUF tile (one logical matmul per block, no PSUM
    accumulation anywhere).
  - pooled layers (conv12, conv22) emit one PSUM tensor with all EVEN
    output positions and one with all ODD positions (two col-tiled
    matmuls per tensor, at array column offsets 0 and 64, which the PE
    runs concurrently). The maxpool is then a single partition-aligned
    elementwise max of the two relu'd tensors.
  - maxpool commutes with relu and the per-channel bias add, so bias+relu
    are applied during PSUM evacuation (ACT activation with per-partition
    bias, or DVE tensor_scalar (x+bias) max 0), and the pool runs on the
    evacuated SBUF tiles (GPSIMD tensor_max, keeping DVE free).

The input is pre-transposed/chunked on the host to [n_tiles, 24, NT] per
core (DRAM partition strides must stay <= 32KB), and the output is
produced as [n_tiles, 24, NT] and reassembled on the host. All weights
and biases ship as two packed blobs (one DMA each at kernel start).
"""

import numpy as np

import concourse.bass as bass
import concourse.tile as tile
import concourse.mybir as mybir
from concourse import bacc
from concourse.bass_utils import run_bass_kernel_spmd

# ---------------------------------------------------------------- config
N_CORES = 8
S, B, L = 512, 256, 24
SB = S * B
CORE_N = SB // N_CORES  # 16384

# compute dtype for matmul operands / intermediate activations:
#   "fp16"  : float16 operands, fp32 PSUM accumulate, NT=1024
#   "fp32r" : fp32 bits, PE in float32r mode (full rate at N>=256), NT=512
#   "fp32"  : exact fp32 (PE 4x slower), NT=512
COMPUTE = "fp16"


def _cfg(compute):
    if compute == "fp16":
        return dict(dt=mybir.dt.float16, np_dt=np.float16, nt=1024, mm_cast=None)
    if compute == "fp32r":
        return dict(
            dt=mybir.dt.float32, np_dt=np.float32, nt=512, mm_cast=mybir.dt.float32r
        )
    if compute == "fp32":
        return dict(dt=mybir.dt.float32, np_dt=np.float32, nt=512, mm_cast=None)
    raise ValueError(compute)


# ------------------------------------------------- host weight transforms
#
# Feature row orderings (all l-major, c-minor):
#   h1 block A: rows (l, c)  l in [0,13), c in [0,8)   -> 104 rows
#   h1 block B: rows (l, c)  l in [11,24)              -> 104 rows
#   pooled h2:  rows [lp 0..5 x8ch | 16 pad | lp 6..11 x8ch] = 112
#   h3 block A: rows (l, c16) l in [0,7)               -> 112 rows
#   h3 block B: rows (l-5, c16) l in [5,12)            -> 112 rows
#   pooled h4:  rows [lp 0..2 x16ch | 16 pad | lp 3..5 x16ch] = 112
#   out: rows j in [0,24)
#
# Pooled-layer PSUM tensors (parity split): psP = even l_out, psQ = odd,
# each with rows [first-half (48) | 16 pad | second-half (48)]; pool =
# elementwise max(relu(psP+b), relu(psQ+b)).

def _band_first(w, l_ins, l_outs, cin, cout):
    """Dense banded matrix [len(l_ins)*cin, len(l_outs)*cout] for a k=3
    'same' conv, rows (l_in, ci) l-major, cols (l_out, co) l-major."""
    K = len(l_ins) * cin
    M = len(l_outs) * cout
    W = np.zeros((K, M), np.float32)
    for ki, li in enumerate(l_ins):
        for ci in range(cin):
            for mo, lo in enumerate(l_outs):
                d = li - lo + 1
                if 0 <= d < 3:
                    for co in range(cout):
                        W[ki * cin + ci, mo * cout + co] = w[co, ci, d]
    return W


def _pad48(W):
    """Insert 16 zero rows at row 48 (pooled tensors carry a pad block)."""
    return np.concatenate([W[:48], np.zeros((16,) + W.shape[1:], W.dtype), W[48:]], 0)


def _band_parity(w, l_ins, l_out_base, half_l, cin, cout):
    """Banded matrix with parity-grouped output: cols = par*64 + lp*cout +
    co, l_out = l_out_base + 2*lp + par (even block rows 0:48, odd block
    rows 64:112, pads 48:64 and 112:128 zeroed by the matmul)."""
    K = len(l_ins) * cin
    W = np.zeros((K, 128), np.float32)
    for ki, li in enumerate(l_ins):
        for ci in range(cin):
            for par in range(2):
                for lp in range(half_l):
                    lo = l_out_base + 2 * lp + par
                    d = li - lo + 1
                    if 0 <= d < 3:
                        for co in range(cout):
                            W[ki * cin + ci, par * 64 + lp * cout + co] = w[co, ci, d]
    return W


def _host_weights(w11, b11, w12, b12, w21, b21, w22, b22, Wl, bl):
    f32 = np.float32
    w11, w12, w21, w22, Wl = (np.asarray(a, f32) for a in (w11, w12, w21, w22, Wl))

    W11A = _band_first(w11, range(0, 24), range(0, 13), 1, 8)      # [24, 104]
    W11B = _band_first(w11, range(0, 24), range(11, 24), 1, 8)     # [24, 104]
    W12A = _band_parity(w12, range(0, 13), 0, 6, 8, 8)             # [104, 128]
    W12B = _band_parity(w12, range(11, 24), 12, 6, 8, 8)           # [104, 128]
    W21A = _pad48(_band_first(w21, range(0, 12), range(0, 7), 8, 16))   # [112, 112]
    W21B = _pad48(_band_first(w21, range(0, 12), range(5, 12), 8, 16))  # [112, 112]
    W22A = _band_parity(w22, range(0, 7), 0, 3, 16, 16)            # [112, 128]
    W22B = _band_parity(w22, range(5, 12), 6, 3, 16, 16)           # [112, 128]
    # torch flatten feature = c*6 + lp ; h4 row = lp*16 + c (plus pad48)
    WLIN = np.zeros((96, 24), f32)
    for lp in range(6):
        for c in range(16):
            WLIN[lp * 16 + c, :] = Wl[:, c * 6 + lp]
    WLIN = _pad48(WLIN)                                            # [112, 24]

    return {
        "w11a": W11A, "w11b": W11B, "w12a": W12A, "w12b": W12B,
        "w21a": W21A, "w21b": W21B, "w22a": W22A, "w22b": W22B,
        "wlin": WLIN,
        "b11v": np.tile(np.asarray(b11, f32), 13).reshape(104, 1),
        "b12v": np.tile(np.asarray(b12, f32), 16).reshape(128, 1),
        "b21v": np.tile(np.asarray(b21, f32), 7).reshape(112, 1),
        "b22v": np.tile(np.asarray(b22, f32), 8).reshape(128, 1),
        "blv": np.asarray(bl, f32).reshape(24, 1),
    }


# weight blob layout: (name, K, M) in packing order
_WSPEC = [
    ("w11a", 24, 104), ("w11b", 24, 104),
    ("w12a", 104, 128), ("w12b", 104, 128),
    ("w21a", 112, 112), ("w21b", 112, 112),
    ("w22a", 112, 128), ("w22b", 112, 128),
    ("wlin", 112, 24),
]
_WOFF = {}
_off = 0
for _n, _k, _m in _WSPEC:
    _WOFF[_n] = _off
    _off += _m
WBLOB_COLS = _off  # 904

_BSPEC = [("b11v", 104), ("b12v", 128), ("b21v", 112), ("b22v", 128), ("blv", 24)]
_BOFF = {n: i for i, (n, _) in enumerate(_BSPEC)}


def _pack_blobs(W, np_dt):
    wb = np.zeros((128, WBLOB_COLS), np_dt)
    for n, k, m in _WSPEC:
        assert W[n].shape == (k, m), (n, W[n].shape)
        wb[:k, _WOFF[n]:_WOFF[n] + m] = W[n].astype(np_dt)
    bb = np.zeros((128, len(_BSPEC)), np.float32)
    for n, p in _BSPEC:
        bb[:p, _BOFF[n]] = W[n][:, 0]
    return wb, bb


# ----------------------------------------------------- numpy device model
def emulate(x, np_dt=np.float16, **kw):
    """Pure-numpy emulation of the device dataflow (same banded matrices,
    same orderings, same cast points). Used to validate index math."""
    W = _host_weights(**kw)
    xt = np.ascontiguousarray(x.reshape(-1, L).T).astype(np_dt)  # [24, N]
    n = xt.shape[1]
    c = lambda a: a.astype(np_dt)

    def mm(wname, act):
        return c(W[wname]).astype(np.float32).T @ act.astype(np.float32)

    def relu_b(a, bias):
        return np.maximum(a + bias, 0.0)

    psA, psB = c(mm("w11a", xt)), c(mm("w11b", xt))
    h1a, h1b = c(relu_b(psA, W["b11v"])), c(relu_b(psB, W["b11v"]))
    psC, psD = c(mm("w12a", h1a)), c(mm("w12b", h1b))
    sA, sB = c(relu_b(psC, W["b12v"])), c(relu_b(psD, W["b12v"]))
    h2r = np.concatenate(
        [np.maximum(sA[0:64], sA[64:128]), np.maximum(sB[0:48], sB[64:112])], 0
    )
    psE, psF = c(mm("w21a", h2r)), c(mm("w21b", h2r))
    h3a, h3b = c(relu_b(psE, W["b21v"])), c(relu_b(psF, W["b21v"]))
    psG, psH = c(mm("w22a", h3a)), c(mm("w22b", h3b))
    sG, sH = c(relu_b(psG, W["b22v"])), c(relu_b(psH, W["b22v"]))
    h4r = np.concatenate(
        [np.maximum(sG[0:64], sG[64:128]), np.maximum(sH[0:48], sH[64:112])], 0
    )
    out = mm("wlin", h4r) + W["blv"]  # fp32
    return out.T.reshape(x.shape[0], x.shape[1], 24).astype(np.float32)


# --------------------------------------------------------- device builder
def build_kernel(n_samples, compute=COMPUTE, n_cores=N_CORES):
    cfg = _cfg(compute)
    DT, NT = cfg["dt"], cfg["nt"]
    MMC = cfg["mm_cast"]
    f32 = mybir.dt.float32
    n_tiles = n_samples // NT
    assert n_samples % NT == 0

    nc = bacc.Bacc(
        "TRN2",
        target_bir_lowering=False,
        debug=False,
        enable_asserts=False,
        num_devices=n_cores,
    )

    # tile-chunked 3D input layout keeps DRAM partition strides small
    # (a flat [24, 16384] fp32 needs 64KB strides, which crash the device)
    xt_d = nc.dram_tensor("xt", [n_tiles, 24, NT], DT, kind="ExternalInput").ap()
    wb_d = nc.dram_tensor("wblob", [128, WBLOB_COLS], DT, kind="ExternalInput").ap()
    bb_d = nc.dram_tensor("bblob", [128, len(_BSPEC)], f32,
                          kind="ExternalInput").ap()
    out_d = nc.dram_tensor("out", [n_tiles, 24, NT], f32, kind="ExternalOutput").ap()

    Relu = mybir.ActivationFunctionType.Relu
    Ident = mybir.ActivationFunctionType.Identity
    Add, Max = mybir.AluOpType.add, mybir.AluOpType.max

    def mmop(ap):
        return ap.bitcast(MMC) if MMC is not None else ap

    # matmul fp32 PSUM output must stay inside one 2KB bank -> <=512 cols
    MMN = min(NT, 512)

    with tile.TileContext(nc) as tc:
        with (
            tc.tile_pool(name="consts", bufs=1) as cpool,
            tc.tile_pool(name="xin", bufs=3) as xpool,
            tc.tile_pool(name="acts", bufs=2) as apool,
            tc.tile_pool(name="outs", bufs=2) as opool,
            tc.tile_pool(name="ps", bufs=3, space="PSUM") as pspool,
            tc.tile_pool(name="pslin", bufs=1, space="PSUM") as lpool,
        ):
            wsb = cpool.tile([128, WBLOB_COLS], DT, tag="wblob")
            bsb = cpool.tile([128, len(_BSPEC)], f32, tag="bblob")
            nc.sync.dma_start(wsb[:], wb_d)
            nc.sync.dma_start(bsb[:], bb_d)

            def w(name):
                k, m = next((kk, mm_) for nn, kk, mm_ in _WSPEC if nn == name)
                return mmop(wsb[0:k, _WOFF[name]:_WOFF[name] + m])

            def bias(name):
                p = next(pp for nn, pp in _BSPEC if nn == name)
                return bsb[0:p, _BOFF[name]:_BOFF[name] + 1]

            def mm(out_ps, wname, rhs_sb, rows=None, tile_pos=None):
                o = out_ps if rows is None else out_ps[rows[0]:rows[1], :]
                for j in range(0, NT, MMN):
                    nc.tensor.matmul(o[:, j:j + MMN], w(wname),
                                     mmop(rhs_sb[:, j:j + MMN]),
                                     start=True, stop=True,
                                     tile_position=tile_pos)

            for t in range(n_tiles):
                xt_t = xpool.tile([24, NT], DT, tag="xt")
                nc.sync.dma_start(xt_t[:], xt_d[t])

                # conv11: two banded matmuls, K=24; ACT evacuates
                psA = pspool.tile([104, NT], f32, tag="ps")
                psB = pspool.tile([104, NT], f32, tag="ps")
                mm(psA, "w11a", xt_t)
                mm(psB, "w11b", xt_t)
                h1a = apool.tile([104, NT], DT, tag="h1a")
                h1b = apool.tile([104, NT], DT, tag="h1b")
                nc.scalar.activation(h1a[:], psA[:], Relu, bias=bias("b11v"))
                nc.scalar.activation(h1b[:], psB[:], Relu, bias=bias("b11v"))

                # conv12 (parity-grouped M=128: even l rows 0:48, odd
                # 64:112): ACT evacuates with bias+relu (commutes with max);
                # a small SBUF->SBUF DMA aligns the odd block's partitions,
                # then DVE tensor_max pools
                psC = pspool.tile([128, NT], f32, tag="ps")
                psD = pspool.tile([128, NT], f32, tag="ps")
                mm(psC, "w12a", h1a)
                mm(psD, "w12b", h1b)
                s12a = apool.tile([128, NT], DT, tag="s12a")
                s12b = apool.tile([128, NT], DT, tag="s12b")
                nc.scalar.activation(s12a[:], psC[:], Relu, bias=bias("b12v"))
                nc.scalar.activation(s12b[:], psD[:], Relu, bias=bias("b12v"))
                mv1 = apool.tile([64, NT], DT, tag="mv1")
                mv2 = apool.tile([112, NT], DT, tag="mv2")
                nc.sync.dma_start(mv1[0:64, :], s12a[64:128, :])
                nc.sync.dma_start(mv2[64:112, :], s12b[0:48, :])
                h2r = apool.tile([112, NT], DT, tag="h2r")
                nc.vector.tensor_max(h2r[0:64, :], s12a[0:64, :], mv1[0:64, :])
                nc.vector.tensor_max(h2r[64:112, :], s12b[64:112, :], mv2[64:112, :])

                # conv21: overlapped l-halves; DVE evacuates (bias+relu via
                # tensor_scalar) to balance ACT
                psE = pspool.tile([112, NT], f32, tag="ps")
                psF = pspool.tile([112, NT], f32, tag="ps")
                mm(psE, "w21a", h2r)
                mm(psF, "w21b", h2r)
                h3a = apool.tile([112, NT], DT, tag="h3a")
                h3b = apool.tile([112, NT], DT, tag="h3b")
                nc.vector.tensor_scalar(h3a[:], psE[:], bias("b21v"), 0.0, Add, Max)
                nc.vector.tensor_scalar(h3b[:], psF[:], bias("b21v"), 0.0, Add, Max)

                # conv22: parity-grouped like conv12; evac split ACT/DVE
                psG = pspool.tile([128, NT], f32, tag="ps")
                psH = pspool.tile([128, NT], f32, tag="ps")
                mm(psG, "w22a", h3a)
                mm(psH, "w22b", h3b)
                s22a = apool.tile([128, NT], DT, tag="s22a")
                s22b = apool.tile([128, NT], DT, tag="s22b")
                nc.scalar.activation(s22a[:], psG[:], Relu, bias=bias("b22v"))
                nc.vector.tensor_scalar(s22b[:], psH[:], bias("b22v"), 0.0, Add, Max)
                mv3 = apool.tile([64, NT], DT, tag="mv3")
                mv4 = apool.tile([112, NT], DT, tag="mv4")
                nc.sync.dma_start(mv3[0:64, :], s22a[64:128, :])
                nc.sync.dma_start(mv4[64:112, :], s22b[0:48, :])
                h4r = apool.tile([112, NT], DT, tag="h4r")
                nc.vector.tensor_max(h4r[0:64, :], s22a[0:64, :], mv3[0:64, :])
                nc.vector.tensor_max(h4r[64:112, :], s22b[64:112, :], mv4[64:112, :])

                # linear 96 -> 24, fp32 out; ACT evac with bias
                psI = lpool.tile([24, NT], f32, tag="pslin")
                mm(psI, "wlin", h4r)
                osb = opool.tile([24, NT], f32, tag="osb")
                nc.scalar.activation(osb[:], psI[:], Ident, bias=bias("blv"))
                nc.sync.dma_start(out_d[t], osb[:])

    nc.compile()
    return nc


# ------------------------------------------------------------- entry point
def _prep_in_maps(x, weights, compute=COMPUTE):
    cfg = _cfg(compute)
    np_dt = cfg["np_dt"]
    nt = cfg["nt"]
    W = _host_weights(**weights)
    wb, bb = _pack_blobs(W, np_dt)
    xt = np.ascontiguousarray(x.reshape(SB, L).T).astype(np_dt)  # [24, SB]
    in_maps = []
    for c in range(N_CORES):
        xc = xt[:, c * CORE_N:(c + 1) * CORE_N]  # [24, CORE_N]
        in_maps.append({
            "xt": np.ascontiguousarray(
                xc.reshape(24, CORE_N // nt, nt).transpose(1, 0, 2)
            ),
            "wblob": wb,
            "bblob": bb,
        })
    return in_maps


def kernel(x, w11, b11, w12, b12, w21, b21, w22, b22, Wl, bl):
    weights = dict(w11=w11, b11=b11, w12=w12, b12=b12, w21=w21, b21=b21,
                   w22=w22, b22=b22, Wl=Wl, bl=bl)
    x = np.asarray(x, np.float32)
    nc = build_kernel(CORE_N, COMPUTE)
    in_maps = _prep_in_maps(x, weights, COMPUTE)
    res = run_bass_kernel_spmd(nc, in_maps, list(range(N_CORES)))
    outs = [
        res.results[c]["out"].transpose(1, 0, 2).reshape(24, CORE_N)
        for c in range(N_CORES)
    ]
    full = np.concatenate(outs, axis=1)  # [24, SB]
    return np.ascontiguousarray(full.T).reshape(S, B, 24).astype(np.float32)
